# revision 1
# baseline (speedup 1.0000x reference)
"""Trainium2 Bass kernel for nn_Connectivity3D (gnn_message_passing).

Pipeline (per the reference):
  PointNet pointwise MLP (6->64->128->256, BN folded into weights) over
  8192 parts x 512 points, max-pool over points -> feat [8192, 256],
  object embedding, 2 GCN layers over dense intra-object edges, pairwise
  connectivity head, scatter into [512, 16, 16].

Key algebraic facts used (exact for the dense intra-object edge structure,
where every object has all K*(K-1) directed edges):
  * deg == 16 for every node, so each GCN layer output is the per-object
    mean of x @ W (plus bias) -- constant across the 16 nodes of an object.
  * Therefore the head input pair = concat([z_obj, z_obj]) is identical for
    every edge of an object, and out[obj] = c_obj * (1 - I).
  * Only the per-object mean of emb is ever needed downstream.

Sharding: data-parallel over objects; core k handles objects [64k, 64k+64)
(parts [1024k, 1024k+1024)). Weights replicated.

On-chip layout: feature-major ("orientation A") -- features on partitions,
points on the free dim.  The input is pre-transposed on the host into 4
"strips" per core (strip i = parts [256i, 256i+256) of the core), with
features of strip i on SBUF partitions [32i, 32i+6).  L1 runs as 4
concurrent row/col-tiled matmuls (K=6, M=64), L2 as row-tiled K=64 pairs,
L3 as full-array K=128 matmuls, all in fp32r (1 cycle/row at N>=512).
The max-pool is a DVE reduce_max straight out of PSUM.
"""

import numpy as np

NUM_OBJ = 512
K = 16
N_PARTS = NUM_OBJ * K        # 8192
P = 512                      # points per part
NCORES = 8
NLOC = N_PARTS // NCORES     # 1024 parts per core
OBJ_LOC = NLOC // K          # 64 objects per core
NSTRIP = 4
NSTREAM = 8                           # 2 point-streams per strip (block-diag K=12)
PARTS_PER_STREAM = NLOC // NSTREAM    # 128
COLS_PER_STRIP = PARTS_PER_STREAM * P # 65536 (each col = one point of 2 streams)
CHUNK_PARTS = 2              # parts per strip per compute chunk
XCHUNK_PARTS = 4             # parts per strip per DMA chunk

USE_F32R = True
DEBUG_FT = False
BENCH_TINY_OUT = False

_prog_cache = {}


def _build_program():
    import concourse.bass as bass
    import concourse.mybir as mybir
    import concourse.tile as tile
    from concourse import bacc
    from contextlib import ExitStack

    f32 = mybir.dt.float32
    f32r = mybir.dt.float32r
    RELU = mybir.ActivationFunctionType.Relu
    IDENT = mybir.ActivationFunctionType.Identity
    TANH = mybir.ActivationFunctionType.Tanh
    AXX = mybir.AxisListType.X

    fbig = f32r if USE_F32R else f32

    nc = bacc.Bacc(trn_type="TRN2", target_bir_lowering=False)

    # ---- DRAM IO ----
    xt_d = nc.dram_tensor("xt", [NSTRIP, 12, COLS_PER_STRIP], fbig, kind="ExternalInput")
    w1_d = nc.dram_tensor("w1r", [128, 128], fbig, kind="ExternalInput")
    b1_d = nc.dram_tensor("b1s", [128, 1], f32, kind="ExternalInput")
    w2_d = nc.dram_tensor("w2r", [128, 128], fbig, kind="ExternalInput")
    b2_d = nc.dram_tensor("b2s", [128, 1], f32, kind="ExternalInput")
    w3_d = nc.dram_tensor("w3s", [128, 256], fbig, kind="ExternalInput")
    wet_d = nc.dram_tensor("wet", [128, 512], f32, kind="ExternalInput")
    bet_d = nc.dram_tensor("bet", [128, 2], f32, kind="ExternalInput")
    wg1_d = nc.dram_tensor("wg1t", [128, 512], f32, kind="ExternalInput")
    bg1_d = nc.dram_tensor("bg1s", [128, 2], f32, kind="ExternalInput")
    wg2_d = nc.dram_tensor("wg2t", [128, 512], f32, kind="ExternalInput")
    bg2_d = nc.dram_tensor("bg2s", [128, 2], f32, kind="ExternalInput")
    wc1_d = nc.dram_tensor("wc1t", [128, 512], f32, kind="ExternalInput")
    bc1_d = nc.dram_tensor("bc1s", [128, 2], f32, kind="ExternalInput")
    wc2_d = nc.dram_tensor("wc2t", [128, 512], f32, kind="ExternalInput")
    bc2_d = nc.dram_tensor("bc2s", [128, 2], f32, kind="ExternalInput")
    wc3_d = nc.dram_tensor("wc3t", [128, 2], f32, kind="ExternalInput")
    bc3_d = nc.dram_tensor("bc3s", [1, 1], f32, kind="ExternalInput")
    msk_d = nc.dram_tensor("mask", [1, 256], f32, kind="ExternalInput")
    out_kind = "Internal" if BENCH_TINY_OUT else "ExternalOutput"
    out_d = nc.dram_tensor("out", [OBJ_LOC, 256], f32, kind=out_kind)
    bench_d = (nc.dram_tensor("bench_out", [1, 4], f32, kind="ExternalOutput")
               if BENCH_TINY_OUT else None)
    ftd_d = [nc.dram_tensor(f"ftdump{h}", [128, NLOC], f32, kind="ExternalOutput")
             for h in range(2)] if DEBUG_FT else None

    with tile.TileContext(nc) as tc, ExitStack() as ctx:
        wp = ctx.enter_context(tc.tile_pool(name="wp", bufs=1))
        xp = ctx.enter_context(tc.tile_pool(name="xp", bufs=3))
        h1p = ctx.enter_context(tc.tile_pool(name="h1p", bufs=4))
        h2p = ctx.enter_context(tc.tile_pool(name="h2p", bufs=10))
        ftp = ctx.enter_context(tc.tile_pool(name="ftp", bufs=1))
        s2p = ctx.enter_context(tc.tile_pool(name="s2p", bufs=2))
        p1 = ctx.enter_context(tc.tile_pool(name="p1", bufs=2, space="PSUM"))
        p2 = ctx.enter_context(tc.tile_pool(name="p2", bufs=2, space="PSUM"))
        p3 = ctx.enter_context(tc.tile_pool(name="p3", bufs=2, space="PSUM"))

        # ---- load weights/constants into SBUF ----
        def wload(dram, shape, dt=f32):
            t = wp.tile(shape, dt, tag=dram.name, name=dram.name + "_s")
            nc.sync.dma_start(out=t[:], in_=dram[:])
            return t

        w1s = wload(w1_d, [128, 128], dt=fbig)
        b1s = wload(b1_d, [128, 1])
        w2s = wload(w2_d, [128, 128], dt=fbig)
        b2s = wload(b2_d, [128, 1])
        w3s = wload(w3_d, [128, 256], dt=fbig)
        wets = wload(wet_d, [128, 512])
        bets = wload(bet_d, [128, 2])
        wg1s = wload(wg1_d, [128, 512])
        bg1s = wload(bg1_d, [128, 2])
        wg2s = wload(wg2_d, [128, 512])
        bg2s = wload(bg2_d, [128, 2])
        wc1s = wload(wc1_d, [128, 512])
        bc1s = wload(bc1_d, [128, 2])
        wc2s = wload(wc2_d, [128, 512])
        bc2s = wload(bc2_d, [128, 2])
        wc3s = wload(wc3_d, [128, 2])
        bc3s = wload(bc3_d, [1, 1])
        msks = wload(msk_d, [1, 256])

        # feat^T accumulators: feats 0-127 / 128-255  x  all parts of the core
        ft = [ftp.tile([128, NLOC], f32, tag=f"ft{h}", name=f"ft{h}") for h in range(2)]

        # ---- main PointNet loop ----
        n_xchunks = COLS_PER_STRIP // (XCHUNK_PARTS * P)     # 32
        cc_per_x = XCHUNK_PARTS // CHUNK_PARTS               # 2
        XCOLS = XCHUNK_PARTS * P                             # 2048

        for xc in range(n_xchunks):
            xt_t = xp.tile([128, XCOLS], fbig, tag="xt", name="xt_t")
            for i in range(NSTRIP):
                nc.sync.dma_start(
                    out=xt_t[32 * i : 32 * i + 12, :],
                    in_=xt_d[i, :, xc * XCOLS : (xc + 1) * XCOLS],
                )
            for cc in range(cc_per_x):
                c_glob = xc * cc_per_x + cc                  # global chunk id
                h2t = {}
                for sub in range(CHUNK_PARTS):
                    xsl = slice(cc * CHUNK_PARTS * P + sub * P,
                                cc * CHUNK_PARTS * P + (sub + 1) * P)
                    for i in range(NSTRIP):
                        # L1: block-diag K=12 -> 2 streams stacked in M
                        p1t = p1.tile([128, P], f32, tag="p1", name="p1t")
                        nc.tensor.matmul(
                            p1t[:],
                            lhsT=w1s[32 * i : 32 * i + 12, :],
                            rhs=xt_t[32 * i : 32 * i + 12, xsl],
                            tile_position=(32 * i, 0),
                        )
                        h1t = h1p.tile([128, P], fbig, tag="h1", name="h1t")
                        nc.scalar.activation(h1t[:], p1t[:], RELU, bias=b1s[:, 0:1])
                        # L2: row-tiled K=64, one per stream
                        for j in range(2):
                            r0 = 64 * j
                            p2t = p2.tile([128, P], f32, tag="p2", name="p2t")
                            nc.tensor.matmul(
                                p2t[:],
                                lhsT=w2s[r0 : r0 + 64, :],
                                rhs=h1t[r0 : r0 + 64, :],
                                tile_position=(r0, 0),
                            )
                            h2 = h2p.tile([128, P], fbig, tag="h2", name="h2t")
                            nc.scalar.activation(h2[:], p2t[:], RELU, bias=b2s[:, 0:1])
                            h2t[(2 * i + j, sub)] = h2
                # L3 + max-pool: per stream, per feature-half
                for s in range(NSTREAM):
                    for h in range(2):
                        p3t = p3.tile([128, CHUNK_PARTS * P], f32, tag="p3", name="p3t")
                        for sub in range(CHUNK_PARTS):
                            nc.tensor.matmul(
                                p3t[:, sub * P : (sub + 1) * P],
                                lhsT=w3s[:, 128 * h : 128 * h + 128],
                                rhs=h2t[(s, sub)][:],
                            )
                        pc0 = PARTS_PER_STREAM * s + CHUNK_PARTS * c_glob
                        nc.vector.reduce_max(
                            ft[h][:, pc0 : pc0 + CHUNK_PARTS],
                            p3t[:].rearrange("p (s q) -> p s q", q=P),
                            axis=AXX,
                        )

        # ---- stage 2: per-object mean -> GCN x2 -> head -> output ----
        def dense256(win, bin_, src, func):
            """[128,64] x2 halves: out_h = func(sum_k win[k,h].T @ src[k] + b_h)"""
            outs = []
            for h in range(2):
                pe = p1.tile([128, OBJ_LOC], f32, tag="p1", name="pe")
                for k in range(2):
                    nc.tensor.matmul(
                        pe[:],
                        lhsT=win[:, 256 * k + 128 * h : 256 * k + 128 * h + 128],
                        rhs=src[k][:],
                        start=(k == 0),
                        stop=(k == 1),
                    )
                o = s2p.tile([128, OBJ_LOC], f32, tag=f"s2_{id(win)}_{h}", name=f"s2o{h}")
                nc.scalar.activation(o[:], pe[:], func, bias=bin_[:, h : h + 1])
                outs.append(o)
            return outs

        sfeat = []
        for h in range(2):
            sf = s2p.tile([128, OBJ_LOC], f32, tag=f"sf{h}", name=f"sf{h}")
            nc.vector.reduce_sum(
                sf[:], ft[h][:].rearrange("p (o k) -> p o k", k=K), axis=AXX
            )
            sfeat.append(sf)

        memb = dense256(wets, bets, sfeat, IDENT)      # mean emb per object
        x1 = dense256(wg1s, bg1s, memb, RELU)          # GCN layer 1
        z = dense256(wg2s, bg2s, x1, IDENT)            # GCN layer 2
        c1 = dense256(wc1s, bc1s, z, RELU)             # head L1 (folded concat)
        c2 = dense256(wc2s, bc2s, c1, RELU)            # head L2

        ps = p2.tile([1, OBJ_LOC], f32, tag="p2", name="ps")
        for k in range(2):
            nc.tensor.matmul(
                ps[:], lhsT=wc3s[:, k : k + 1], rhs=c2[k][:],
                start=(k == 0), stop=(k == 1),
            )
        c_sb = s2p.tile([1, OBJ_LOC], f32, tag="c_sb", name="c_sb")
        nc.scalar.activation(c_sb[:], ps[:], TANH, bias=bc3s[:, 0:1])

        # out[obj, ij] = c_obj * mask[ij] via K=1 matmul broadcast
        po = p3.tile([OBJ_LOC, 256], f32, tag="p3", name="po")
        nc.tensor.matmul(po[:], lhsT=c_sb[:], rhs=msks[:])
        out_sb = s2p.tile([OBJ_LOC, 256], f32, tag="out_sb", name="out_sb")
        nc.scalar.copy(out_sb[:], po[:])
        nc.sync.dma_start(out=out_d[:], in_=out_sb[:])
        if BENCH_TINY_OUT:
            nc.sync.dma_start(out=bench_d[:], in_=out_sb[0:1, 0:4])
        if DEBUG_FT:
            for h in range(2):
                nc.sync.dma_start(out=ftd_d[h][:], in_=ft[h][:])

    nc.compile()
    return nc


def _prep_inputs(inputs):
    """Fold BN/bias algebra on the host; build per-core input maps."""
    g = {k: np.asarray(v, np.float32) for k, v in inputs.items()
         if not k.startswith("edge")}

    W1f = g["W1"] * g["g1"][None, :]
    b1f = g["b1"] * g["g1"] + g["bt1"]
    W2f = g["W2"] * g["g2"][None, :]
    b2f = g["b2"] * g["g2"] + g["bt2"]
    W3f = g["W3"] * g["g3"][None, :]
    b3f = g["b3"] * g["g3"] + g["bt3"]

    wet = g["We"] / np.float32(K)
    bet = b3f @ g["We"] + g["be"]          # absorbs the L3 bias via the mean
    wc1f = g["Wc1"][:256] + g["Wc1"][256:]  # pair = [z, z] fold

    def tile256(W):
        return np.ascontiguousarray(
            W.reshape(2, 128, 2, 128).transpose(1, 0, 2, 3).reshape(128, 512)
        )

    def bias2(b):
        return np.ascontiguousarray(b.reshape(2, 128).T)

    w1r = np.zeros((128, 128), np.float32)
    for i in range(NSTRIP):
        w1r[32 * i : 32 * i + 6, 0:64] = W1f
        w1r[32 * i + 6 : 32 * i + 12, 64:128] = W1f
    b1s = np.ascontiguousarray(np.concatenate([b1f, b1f])[:, None])
    w2r = np.ascontiguousarray(np.vstack([W2f, W2f]))
    b2s = np.ascontiguousarray(b2f[:, None])
    w3s = np.ascontiguousarray(W3f)

    mask = (1.0 - np.eye(K, dtype=np.float32)).reshape(1, 256)

    shared = {
        "w1r": w1r, "b1s": b1s, "w2r": w2r, "b2s": b2s, "w3s": w3s,
        "wet": tile256(wet), "bet": bias2(bet),
        "wg1t": tile256(g["Wg1"]), "bg1s": bias2(g["bg1"]),
        "wg2t": tile256(g["Wg2"]), "bg2s": bias2(g["bg2"]),
        "wc1t": tile256(wc1f), "bc1s": bias2(g["bc1"]),
        "wc2t": tile256(g["Wc2"]), "bc2s": bias2(g["bc2"]),
        "wc3t": np.ascontiguousarray(g["Wc3"].reshape(2, 128).T),
        "bc3s": g["bc3"].reshape(1, 1).astype(np.float32),
        "mask": mask,
    }

    pcls = np.asarray(inputs["pcls_arr"], np.float32)
    in_maps = []
    for k in range(NCORES):
        pc = pcls[k * NLOC : (k + 1) * NLOC]                   # [1024, 512, 6]
        xt = np.ascontiguousarray(
            pc.reshape(NSTRIP, 2, PARTS_PER_STREAM, P, 6)
            .transpose(0, 1, 4, 2, 3)
            .reshape(NSTRIP, 12, COLS_PER_STRIP)
        )
        m = dict(shared)
        m["xt"] = xt
        in_maps.append(m)
    return in_maps


def _get_prog():
    if "nc" not in _prog_cache:
        _prog_cache["nc"] = _build_program()
    return _prog_cache["nc"]


def _run(inputs, trace=False, **kw):
    from concourse.bass_utils import run_bass_kernel_spmd

    nc = _get_prog()
    in_maps = _prep_inputs(inputs)
    res = run_bass_kernel_spmd(
        nc, in_maps, core_ids=list(range(NCORES)), trace=trace, **kw
    )
    outs = [r["out"].reshape(OBJ_LOC, K, K) for r in res.results]
    full = np.concatenate(outs, axis=0).astype(np.float32)
    return full, res


def kernel(**inputs) -> np.ndarray:
    out, _ = _run(inputs, trace=False)
    return out


def bench(inputs, **kw):
    """Run with profiling; returns (output, BassKernelResults)."""
    return _run(inputs, trace=True, **kw)

